# revision 1
# baseline (speedup 1.0000x reference)
"""ChannelAttention kernel for Trainium2 (Bass/Tile), 8-core SPMD.

Reference (per sample b, xf = x[b] as [C=256, N=16384]):
    F  = W_f @ xf                      [50, N]
    S  = softmax(F @ xf^T, axis=C)     [50, 256]
    E  = S^T @ F ; out = W_beta @ E + xf

Key algebraic restructure: out = (W_beta @ S^T) @ F + x = M @ F + x where
M = W_beta @ S^T is a tiny [256, 50] matrix computed once per sample after
softmax — the big E tensor is never materialized.

Sharding: 8 cores = 4 samples x 2 spatial halves (x[b][:, h*8192:(h+1)*8192]).
The only cross-core coupling is the S contraction over N: partial S per
core, AllReduce within pairs [[0,1],[2,3],[4,5],[6,7]] (51 KB), then local.

Measured cost structure on this part: each PE instruction costs roughly
max(~240 ns issue/sync overhead, moving-stream time), so the design
minimizes PE instruction count and keeps streams under the overhead:
  - F = W_f x and the partial-S contraction run with fp32r operand views
    (full-rate streaming; storage stays fp32 so the PE transposes and the
    residual path remain bit-exact). Only the two logit matmuls see the
    ~1e-3-level fp32r operand rounding; headroom to the 2e-2 gate is ~10x.
  - x^T and F^T come from PE transposes (exact, 2 cyc/row fp32).
  - phase 3 out = M F + x runs in bf16 with DVE/ACT alternating PSUM
    evacuation + residual adds, 1 MiB stores on alternating HWDGE rings.

Software pipelining across n_iters: iteration it+1's x loads and its first
HOIST_NT phase-1 n-tiles are EMITTED before iteration it's phase 3. Engine
queues execute in order, so the PE chews on next-iteration work during
it's AllReduce+softmax bubble, and next-iteration load DMAs overlap this
iteration's store DMAs on the two HWDGE rings.

n_iters > 1 repeats the whole dataflow (including DMAs and the collective)
inside one NEFF — used by test.py to measure per-iteration HW time by
differencing, since NTFF profiling is unavailable under axon.
"""

import os
import numpy as np
from contextlib import ExitStack

import concourse.bass as bass
import concourse.tile as tile
from concourse import mybir
from concourse.bass_utils import run_bass_kernel_spmd
from concourse.masks import make_identity

B, C, O = 4, 256, 50
N = 128 * 128            # 16384 spatial positions
NCORES = 8
NH = N // 2              # 8192 per core
NT = 512                 # phase-1/3 n-tile
NSUB = 128               # transpose sub-tile
NTILES = NH // NT        # 16
XG = 2048                # x DMA group (1 MiB per chunk DMA)
HOIST_NT = 4             # phase-1 n-tiles of it+1 emitted before phase3(it)
F32 = mybir.dt.float32
F32R = mybir.dt.float32r
BF16 = mybir.dt.bfloat16
ActF = mybir.ActivationFunctionType

_CACHE: dict = {}
last_results = None  # exposes BassKernelResults to test.py

# This walrus build rejects instructions carrying more than one embedded
# semaphore wait ("Too many sync wait commands" in setupSyncWait). After
# Tile finishes sem assignment, hoist excess waits onto InstNoOp
# instructions inserted before the offender on the same engine — engine
# program order makes the split semantically identical.
_MAX_WAITS = 1


def _split_multiwait(nc) -> int:
    n_nops = 0
    for fn in nc.m.functions:
        for blk in fn.blocks:
            out = []
            changed = False
            for inst in list(blk.instructions):
                si = inst.sync_info
                waits = list(si.on_wait) if si is not None and si.on_wait else []
                if len(waits) > _MAX_WAITS:
                    keep = waits[-_MAX_WAITS:]
                    hoist = waits[:-_MAX_WAITS]
                    for i in range(0, len(hoist), _MAX_WAITS):
                        nop = mybir.InstNoOp(name=f"I-waitnop-{n_nops}")
                        n_nops += 1
                        nop.engine = inst.engine
                        nop.sync_info = mybir.SyncInfo(
                            on_wait=hoist[i:i + _MAX_WAITS], on_update=[]
                        )
                        out.append(nop)
                    changed = True
                    inst.sync_info = mybir.SyncInfo(
                        on_wait=keep,
                        on_update=list(si.on_update) if si.on_update else [],
                    )
                out.append(inst)
            if changed:
                blk.instructions = out
    return n_nops


def _build_nc(n_iters: int = 1, debug: bool = False,
              single: bool = False, skip_cc: bool = False) -> bass.Bass:
    # single=True: 1-device build with the AllReduce replaced by a local
    # DRAM copy — only for TimelineSim cost-model analysis (single-core).
    # skip_cc=True: 8-core build, AllReduce replaced by local copy — timing
    # ablation only (results wrong by the missing cross-half reduction).
    nc = bass.Bass(num_devices=1 if single else NCORES)

    # f32r dram dtypes: same bits as f32 (numpy side binds float32); lets
    # the sync/scalar rings load straight into f32r tiles without a "cast"
    xs = nc.dram_tensor("xs", [2, 128, NH], F32R, kind="ExternalInput")
    wft = nc.dram_tensor("wft", [2, 128, O], F32R, kind="ExternalInput")
    wbt = nc.dram_tensor("wbt", [2, 128, C], F32R, kind="ExternalInput")
    out = nc.dram_tensor("out", [2, 128, NH], F32, kind="ExternalOutput")
    if debug:
        dbg_s = nc.dram_tensor("dbg_s", [O, C], F32, kind="ExternalOutput")
        dbg_m = nc.dram_tensor("dbg_m", [O, C], F32, kind="ExternalOutput")
        dbg_f = nc.dram_tensor("dbg_f", [O, NH], BF16, kind="ExternalOutput")

    n_xgroups = NH // XG          # 4 DMA groups per c-chunk
    subs = NT // NSUB             # 4 transpose sub-tiles per n-tile

    with tile.TileContext(nc) as tc, ExitStack() as ctx:
        const = ctx.enter_context(tc.tile_pool(name="const", bufs=1))
        xpool = ctx.enter_context(tc.tile_pool(name="x", bufs=2))
        fpool = ctx.enter_context(tc.tile_pool(name="f", bufs=2))
        stage = ctx.enter_context(tc.tile_pool(name="stage", bufs=4))
        spool = ctx.enter_context(tc.tile_pool(name="smax", bufs=2))
        opool = ctx.enter_context(tc.tile_pool(name="o", bufs=2))
        dram = ctx.enter_context(tc.tile_pool(name="dram", bufs=2, space="DRAM"))
        # PSUM: exactly 8 banks. f 1 + tr 2 + ftr 1 + s 1 + o 2 + m 1.
        psF = ctx.enter_context(tc.tile_pool(name="psF", bufs=1, space="PSUM"))
        psT = ctx.enter_context(tc.tile_pool(name="psT", bufs=2, space="PSUM"))
        psFT = ctx.enter_context(tc.tile_pool(name="psFT", bufs=1, space="PSUM"))
        psS = ctx.enter_context(tc.tile_pool(name="psS", bufs=1, space="PSUM"))
        psO = ctx.enter_context(tc.tile_pool(name="psO", bufs=2, space="PSUM"))
        psM = ctx.enter_context(tc.tile_pool(name="psM", bufs=1, space="PSUM"))

        # weights first (tiny), then x loads can stream
        ident = const.tile([128, 128], F32, tag="ident")
        wft_sb = []
        wbt_sb = []
        for ci in range(2):
            t = const.tile([128, O], F32R, tag=f"wft{ci}")
            nc.gpsimd.dma_start(t[:], wft[ci])
            wft_sb.append(t)
            t = const.tile([128, C], F32R, tag=f"wbt{ci}")
            nc.gpsimd.dma_start(t[:], wbt[ci])
            wbt_sb.append(t)
        make_identity(nc, ident[:])

        st: dict = {}

        def emit_loads(it: int):
            x_sb = [[None] * n_xgroups for _ in range(2)]
            for g in range(n_xgroups):
                for ci in range(2):
                    # only group 0/1 need double buffers (the cross-iter
                    # hoist); groups 2-3 reload in a wavefront behind the
                    # phase-3 residual reads of the previous iteration
                    t = xpool.tile([128, XG], F32R, tag=f"x{ci}_{g}",
                                   bufs=2 if g < 2 else 1)
                    eng = nc.sync if (g * 2 + ci) % 2 == 0 else nc.scalar
                    eng.dma_start(t[:], xs[ci, :, g * XG:(g + 1) * XG])
                    x_sb[ci][g] = t
            frhs = fpool.tile([O, NH], BF16, tag="Fr")
            st[it] = {"x": x_sb, "frhs": frhs,
                      "xT": {}, "s_ps": None, "nt": 0}

        def xsl(it: int, ci: int, n0: int, w: int, as_f32: bool = False):
            g, loc = divmod(n0, XG)
            assert loc + w <= XG
            ap = st[it]["x"][ci][g][:, loc:loc + w]
            return ap.bitcast(F32) if as_f32 else ap

        def emit_produce(it: int, nt: int):
            # n-tile nt: F matmul (fp32r views), x^T transposes, F^T
            # transposes; evacuations to SBUF stage tiles.
            s = st[it]
            n0 = nt * NT
            f_ps = psF.tile([O, NT], F32, tag="f_ps")
            for ci in range(2):
                nc.tensor.matmul(
                    f_ps[:],
                    wft_sb[ci][:],
                    xsl(it, ci, n0, NT),
                    start=(ci == 0),
                    stop=(ci == 1),
                )
            f_st = stage.tile([O, NT], F32, tag="f_st")
            nc.scalar.activation(f_st[:], f_ps[:], ActF.Copy)
            nc.scalar.activation(s["frhs"][:, n0:n0 + NT], f_ps[:], ActF.Copy)

            # x^T: 8 transposes -> 2 merged PSUM banks -> 2 DVE copies
            # layout [128, 2, 256] = [sub | sub+1] x [ci0 | ci1]
            xT_sb = []
            for half in range(2):
                tr_ps = psT.tile([128, 2, C], F32, tag="tr")
                for s2 in range(2):
                    sn0 = n0 + (half * 2 + s2) * NSUB
                    for ci in range(2):
                        nc.tensor.transpose(
                            tr_ps[:, s2, ci * 128:(ci + 1) * 128],
                            xsl(it, ci, sn0, NSUB, as_f32=True),
                            ident[:],
                        )
                xT = stage.tile([128, 2, C], F32R, tag="xT")
                nc.vector.tensor_copy(xT[:], tr_ps[:])
                xT_sb.append(xT)

            # F^T: 4 transposes -> 1 PSUM bank -> 1 DVE copy
            ftr_ps = psFT.tile([128, subs, O], F32, tag="ftr")
            for sb in range(subs):
                nc.tensor.transpose(
                    ftr_ps[:, sb], f_st[:, sb * NSUB:(sb + 1) * NSUB],
                    ident[:O, :O]
                )
            fT = stage.tile([128, subs, O], F32R, tag="fT")
            nc.vector.tensor_copy(fT[:], ftr_ps[:])
            s["xT"][nt] = (xT_sb, fT)

        def emit_consume(it: int, nt: int):
            # n-tile nt: 4 partial-S matmuls (fp32r views) accumulating in
            # one PSUM bank across the whole iteration.
            s = st[it]
            if s["s_ps"] is None:
                s["s_ps"] = psS.tile([O, C], F32, tag="s_ps", name="s_ps")
            s_ps = s["s_ps"]
            xT_sb, fT = s["xT"].pop(nt)
            for sb in range(subs):
                idx = nt * subs + sb
                nc.tensor.matmul(
                    s_ps[:],
                    fT[:, sb],
                    xT_sb[sb // 2][:, sb % 2],
                    start=(idx == 0),
                    stop=(idx == NTILES * subs - 1),
                )

        def emit_phase1_to(it: int, nt_to: int):
            s = st[it]
            while s["nt"] < nt_to:
                nt = s["nt"]
                emit_produce(it, nt)
                if nt > 0:
                    emit_consume(it, nt - 1)
                s["nt"] = nt + 1
            if nt_to == NTILES:
                emit_consume(it, NTILES - 1)

        def emit_cc(it: int):
            s = st[it]
            s_part = spool.tile([O, C], F32, tag="s_part")
            nc.vector.tensor_copy(s_part[:], s["s_ps"][:])
            cc_in = dram.tile([O, C], F32, tag="cc_in")
            cc_out = dram.tile([O, C], F32, tag="cc_out")
            nc.gpsimd.dma_start(cc_in[:], s_part[:])
            if single or skip_cc:
                nc.gpsimd.dma_start(cc_out[:], cc_in[:])
            else:
                nc.gpsimd.collective_compute(
                    "AllReduce",
                    mybir.AluOpType.add,
                    replica_groups=[[0, 1], [2, 3], [4, 5], [6, 7]],
                    ins=[cc_in.opt()],
                    outs=[cc_out.opt()],
                )
            s_full = spool.tile([O, C], F32, tag="s_full")
            nc.gpsimd.dma_start(s_full[:], cc_out[:])
            s["s_full"] = s_full

        def emit_softmax_m(it: int):
            s = st[it]
            s_full = s["s_full"]
            mx = spool.tile([O, 1], F32, tag="mx")
            nc.vector.tensor_reduce(
                mx[:], s_full[:], axis=mybir.AxisListType.X,
                op=mybir.AluOpType.max,
            )
            nmx = spool.tile([O, 1], F32, tag="nmx")
            nc.vector.tensor_scalar_mul(nmx[:], mx[:], -1.0)
            p_exp = spool.tile([O, C], F32, tag="p_exp")
            ssum = spool.tile([O, 1], F32, tag="ssum")
            nc.scalar.activation(
                p_exp[:], s_full[:], ActF.Exp, bias=nmx[:], accum_out=ssum[:]
            )
            rsum = spool.tile([O, 1], F32, tag="rsum")
            nc.vector.reciprocal(rsum[:], ssum[:])
            p_norm = spool.tile([O, C], F32, tag="p_norm")
            nc.vector.tensor_scalar_mul(p_norm[:], p_exp[:], rsum[:])

            # M^T = S @ W_beta^T  [50, 256]
            st_sb = []
            for ci in range(2):
                stp = psM.tile([128, O], F32, tag="m_seq")
                nc.tensor.transpose(
                    stp[:], p_norm[:, ci * 128:(ci + 1) * 128], ident[:O, :O]
                )
                t = spool.tile([128, O], F32R, tag=f"stsb{ci}")
                nc.vector.tensor_copy(t[:], stp[:])
                st_sb.append(t)
            m_ps = psM.tile([O, C], F32, tag="m_seq")
            for ci in range(2):
                nc.tensor.matmul(
                    m_ps[:], st_sb[ci][:], wbt_sb[ci][:],
                    start=(ci == 0), stop=(ci == 1),
                )
            mT_sb = spool.tile([O, C], BF16, tag="mT")
            nc.vector.tensor_copy(mT_sb[:], m_ps[:])
            s["mT"] = mT_sb
            if debug and it == 0:
                nc.sync.dma_start(dbg_s[:, :], s_full[:])
                m_f32 = spool.tile([O, C], F32, tag="m_f32")
                nc.vector.tensor_copy(m_f32[:], m_ps[:])
                nc.sync.dma_start(dbg_m[:, :], m_f32[:])
                nc.sync.dma_start(dbg_f[:, :], s["frhs"][:])

        def emit_phase3(it: int):
            s = st[it]
            mT = s["mT"]
            frhs = s["frhs"]
            # d outer: 16 consecutive matmuls share the same stationary mT
            # half, minimizing LDWEIGHTS churn on the PE queue
            for d in range(2):
                for np4 in range(NTILES // 4):
                    # 4 n-tiles per 1 MiB store; alternate HWDGE rings
                    o_sb = opool.tile([128, 4 * NT], F32, tag="o_sb")
                    for k in range(4):
                        nt = np4 * 4 + k
                        n0 = nt * NT
                        o_ps = psO.tile([128, NT], F32, tag="o_ps")
                        nc.tensor.matmul(
                            o_ps[:],
                            mT[:, d * 128:(d + 1) * 128],
                            frhs[:, n0:n0 + NT],
                            start=True, stop=True,
                        )
                        osl = o_sb[:, k * NT:(k + 1) * NT]
                        if nt % 2 == 0:
                            # split residual work across engines: ACT
                            # evacuates PSUM, DVE adds x with both operands
                            # in SBUF (2x mode)
                            nc.scalar.activation(osl, o_ps[:], ActF.Copy)
                            nc.vector.tensor_add(
                                osl, osl, xsl(it, d, n0, NT, as_f32=True))
                        else:
                            nc.vector.tensor_add(
                                osl, o_ps[:], xsl(it, d, n0, NT, as_f32=True)
                            )
                    n0 = np4 * 4 * NT
                    (nc.sync if (np4 + d) % 2 == 0 else nc.scalar).dma_start(
                        out[d, :, n0:n0 + 4 * NT], o_sb[:]
                    )

        for it in range(n_iters):
            if it not in st:
                emit_loads(it)
            emit_phase1_to(it, NTILES)
            emit_cc(it)
            if it + 1 < n_iters:
                emit_loads(it + 1)
                emit_phase1_to(it + 1, HOIST_NT)
            emit_softmax_m(it)
            emit_phase3(it)
            st.pop(it)

    _split_multiwait(nc)
    return nc


def _get_nc(fast: bool = False, n_iters: int = 1):
    # `fast` kept for test.py signature compatibility; single precision mode.
    key = ("nc", n_iters)
    if key not in _CACHE:
        _CACHE[key] = _build_nc(n_iters)
    return _CACHE[key]


def _make_in_maps(x, W_f, W_beta):
    xf = np.ascontiguousarray(x.reshape(B, C, N), dtype=np.float32)
    wft = np.ascontiguousarray(W_f.T.reshape(2, 128, O), dtype=np.float32)
    wbt = np.ascontiguousarray(W_beta.T.reshape(2, 128, C), dtype=np.float32)
    in_maps = []
    for c in range(NCORES):
        b, h = divmod(c, 2)
        shard = np.ascontiguousarray(
            xf[b, :, h * NH:(h + 1) * NH].reshape(2, 128, NH)
        )
        in_maps.append({"xs": shard, "wft": wft, "wbt": wbt})
    return in_maps


def kernel(x: np.ndarray, W_f: np.ndarray, W_beta: np.ndarray) -> np.ndarray:
    global last_results
    nc = _get_nc()

    in_maps = _make_in_maps(x, W_f, W_beta)
    res = run_bass_kernel_spmd(nc, in_maps, list(range(NCORES)))
    last_results = res

    outv = np.empty((B, C, N), dtype=np.float32)
    for c in range(NCORES):
        b, h = divmod(c, 2)
        outv[b, :, h * NH:(h + 1) * NH] = res.results[c]["out"].reshape(C, NH)
    return outv.reshape(B, C, 128, 128)

